# revision 1
# baseline (speedup 1.0000x reference)
"""Causal self-attention (B=2, T=2048, C=1024, H=16) on 8 trn2 NeuronCores.

Sharding: core c handles batch b = c // 4 and head-group g = c % 4 (4 heads).
Each core computes its heads' attention and a partial output projection
(rows 256g:256g+256 of w_proj); the host sums the 4 partials per batch and
adds b_proj.

Self-contained: hardcodes all shapes; only needs concourse (on sys.path via
the environment) and numpy.
"""
import numpy as np

import concourse.bacc as bacc
import concourse.tile as tile
import concourse.mybir as mybir
from concourse.bass_utils import run_bass_kernel_spmd

F32 = mybir.dt.float32
BF16 = mybir.dt.bfloat16

B, T, C = 2, 2048, 1024
N_HEAD = 16
D = C // N_HEAD          # 64
SCALE = D ** -0.5
HL = 4                   # heads per core (local)
CL = HL * D              # 256 local qkv feature cols per section
TT = T // 128            # 16 t-tiles of 128
TB = T // 512            # 4 t-blocks of 512
CT = C // 128            # 8 c-tiles (contraction for qkv)
VW = D + 1               # 65: v columns per head (+ ones col for denominator)


def _build():
    nc = bacc.Bacc("TRN2", debug=False)
    x_d = nc.dram_tensor("x", [T, C], F32, kind="ExternalInput").ap()
    w_d = nc.dram_tensor("w", [C, 3 * CL], BF16, kind="ExternalInput").ap()
    b_d = nc.dram_tensor("b", [3 * CL], BF16, kind="ExternalInput").ap()
    wp_d = nc.dram_tensor("wp", [2 * 128, C], BF16, kind="ExternalInput").ap()
    tri2_d = nc.dram_tensor("tri2", [128, 256], BF16, kind="ExternalInput").ap()
    id_d = nc.dram_tensor("ident", [128, 128], F32, kind="ExternalInput").ap()
    out_d = nc.dram_tensor("out", [T, C], F32, kind="ExternalOutput").ap()

    with tile.TileContext(nc) as tc:
        with tc.tile_pool(name="persist", bufs=1) as pers:
            # long-lived tensors
            ident = pers.tile([128, 128], F32)
            nc.sync.dma_start(out=ident, in_=id_d)
            tri2 = pers.tile([128, 256], BF16)
            w_all = pers.tile([128, CT, 3 * CL], BF16)
            b_sb = pers.tile([1, 3 * CL], BF16)
            wp = pers.tile([128, 2, C], BF16)
            ones = pers.tile([1, 512], BF16)
            nc.vector.memset(ones, 1.0)

            # outputs of phase B (persist across phases)
            qkT = pers.tile([128, 4, T], BF16)       # q01 | q23 | k01 | k23
            v_all = pers.tile([128, TT, HL * VW], BF16)
            yT = pers.tile([128, 2, T], BF16)        # per pair: [dA(64) | dB(64)] x t

            # ---- Phase A: load x, transpose to xT ----
            with tc.tile_pool(name="phA", bufs=3) as pa, \
                 tc.tile_pool(name="phA1", bufs=1) as pa1, \
                 tc.tile_pool(name="psA", bufs=2, space="PSUM") as psa:
                xT = pa1.tile([128, CT, T], BF16)
                for ti in range(TT):
                    x_t = pa.tile([128, C], F32)
                    nc.sync.dma_start(out=x_t, in_=x_d[ti * 128:(ti + 1) * 128, :])
                    for gc in range(2):
                        ps_tr = psa.tile([128, 512], F32)
                        for u in range(4):
                            g = 4 * gc + u
                            nc.tensor.transpose(
                                ps_tr[:, u * 128:(u + 1) * 128],
                                x_t[:, g * 128:(g + 1) * 128], ident)
                        dst = xT[:, 4 * gc:4 * gc + 4, ti * 128:(ti + 1) * 128]
                        src = ps_tr.rearrange("p (u q) -> p u q", u=4)
                        if gc == 0:
                            nc.vector.tensor_copy(dst, src)
                        else:
                            nc.scalar.copy(dst, src)

                # ---- Phase B: QKV projections ----
                with tc.tile_pool(name="psB", bufs=3, space="PSUM") as psb:
                    nc.sync.dma_start(out=w_all, in_=w_d.rearrange("(g p) j -> p g j", p=128))
                    nc.sync.dma_start(out=b_sb, in_=b_d.rearrange("(o j) -> o j", o=1))
                    nc.sync.dma_start(out=wp, in_=wp_d.rearrange("(g p) j -> p g j", p=128))
                    nc.sync.dma_start(out=tri2, in_=tri2_d)
                    # qT / kT: out[j, t] = sum_c w[c, j] xT[c, t]  (+ b[j])
                    for jt in range(4):
                        for tb in range(TB):
                            ps = psb.tile([128, 512], F32, tag="qk")
                            for g in range(CT):
                                nc.tensor.matmul(
                                    ps,
                                    w_all[:, g, jt * 128:(jt + 1) * 128],
                                    xT[:, g, tb * 512:(tb + 1) * 512],
                                    start=(g == 0), stop=False)
                            nc.tensor.matmul(
                                ps, b_sb[0:1, jt * 128:(jt + 1) * 128], ones,
                                start=False, stop=True)
                            dst = qkT[:, jt, tb * 512:(tb + 1) * 512]
                            if tb % 2 == 0:
                                nc.vector.tensor_copy(dst, ps)
                            else:
                                nc.scalar.copy(dst, ps)
                    # v natural: out[s, d] = sum_c xT[c, s] w[c, 2CL + d] (+ b)
                    for ti in range(TT):
                        psv = psb.tile([128, CL], F32, tag="v", bufs=2)
                        for g in range(CT):
                            nc.tensor.matmul(
                                psv,
                                xT[:, g, ti * 128:(ti + 1) * 128],
                                w_all[:, g, 2 * CL:3 * CL],
                                start=(g == 0), stop=False)
                        nc.tensor.matmul(
                            psv, ones[0:1, 0:128], b_sb[0:1, 2 * CL:3 * CL],
                            start=False, stop=True)
                        dst = v_all[:, ti, :].rearrange("p (h w) -> p h w", h=HL)[:, :, 0:D]
                        src = psv.rearrange("p (h d) -> p h d", h=HL)
                        if ti % 2 == 0:
                            nc.vector.tensor_copy(dst, src)
                        else:
                            nc.scalar.copy(dst, src)
                    # ones columns of v_aug
                    vones = v_all.rearrange("p t (h w) -> p t h w", h=HL)[:, :, :, D:VW]
                    nc.vector.memset(vones, 1.0)

            # ---- Phase C: attention per head-pair, per t-block ----
            with tc.tile_pool(name="phC", bufs=3) as pc, \
                 tc.tile_pool(name="phCs", bufs=2) as pcs, \
                 tc.tile_pool(name="psST", bufs=2, space="PSUM") as pst, \
                 tc.tile_pool(name="psY", bufs=1, space="PSUM") as psy:
                for p in range(2):
                    for tb in range(TB):
                        n_si = 4 * (tb + 1)
                        ypsA = psy.tile([VW, 512], F32, tag="ypsA")
                        ypsB = psy.tile([VW, 512], F32, tag="ypsB")
                        for si in range(n_si):
                            k = si - 4 * tb
                            col0 = 128 * k if k >= 0 else 0
                            nw = 512 - col0
                            st = pst.tile([128, 1024], F32, tag="st")
                            # S^T = k^T(d,s)^T-contract q^T(d,t); row-packed pair
                            nc.tensor.matmul(
                                st[:, col0:512],
                                qkT[0:64, 2 + p, si * 128:(si + 1) * 128],
                                qkT[0:64, p, tb * 512 + col0:(tb + 1) * 512],
                                tile_position=(0, 0), start=True, stop=True)
                            nc.tensor.matmul(
                                st[:, 512 + col0:1024],
                                qkT[64:128, 2 + p, si * 128:(si + 1) * 128],
                                qkT[64:128, p, tb * 512 + col0:(tb + 1) * 512],
                                tile_position=(64, 0), start=True, stop=True)
                            pt = pc.tile([128, 1024], BF16, tag="pt")
                            st3 = st.rearrange("p (h q) -> p h q", h=2)[:, :, col0:512]
                            pt3 = pt.rearrange("p (h q) -> p h q", h=2)[:, :, col0:512]
                            nc.scalar.activation(
                                pt3, st3, mybir.ActivationFunctionType.Exp,
                                scale=SCALE)
                            if k >= 0:
                                # mask diag strip: cols [col0, col0+128) per head
                                strip = pt.rearrange("p (h q) -> p h q", h=2)[
                                    :, :, col0:col0 + 128]
                                nc.vector.tensor_mul(
                                    strip, strip,
                                    tri2.rearrange("p (h q) -> p h q", h=2))
                            # PV with denominator row (M=65)
                            nc.tensor.matmul(
                                ypsA[:, col0:512],
                                v_all[:, si, 2 * p * VW:(2 * p + 1) * VW],
                                pt[:, col0:512],
                                start=(si == 0), stop=(si == n_si - 1),
                                skip_group_check=True)
                            nc.tensor.matmul(
                                ypsB[:, col0:512],
                                v_all[:, si, (2 * p + 1) * VW:(2 * p + 2) * VW],
                                pt[:, 512 + col0:1024],
                                start=(si == 0), stop=(si == n_si - 1),
                                skip_group_check=True)
                        recipA = pcs.tile([1, 512], F32, tag="recipA")
                        recipB = pcs.tile([1, 512], F32, tag="recipB")
                        nc.vector.reciprocal(recipA, ypsA[64:65, :])
                        nc.vector.reciprocal(recipB, ypsB[64:65, :])
                        recipAb = pcs.tile([1, 512], BF16, tag="recipAb")
                        recipBb = pcs.tile([1, 512], BF16, tag="recipBb")
                        nc.vector.tensor_copy(recipAb, recipA)
                        nc.vector.tensor_copy(recipBb, recipB)
                        bc = pst.tile([128, 512], F32, tag="bc", bufs=1)
                        nc.tensor.matmul(bc[0:64, :], ones[0:1, 0:64], recipAb,
                                         tile_position=(0, 0), start=True, stop=True)
                        nc.tensor.matmul(bc[64:128, :], ones[0:1, 0:64], recipBb,
                                         tile_position=(0, 64), start=True, stop=True)
                        bc_sb = pcs.tile([128, 512], F32, tag="bc_sb")
                        nc.vector.tensor_copy(bc_sb, bc)
                        tsl = slice(tb * 512, (tb + 1) * 512)
                        nc.vector.tensor_mul(
                            yT[0:64, p, tsl], ypsA[0:64, :], bc_sb[0:64, :])
                        nc.vector.tensor_mul(
                            yT[64:128, p, tsl], ypsB[0:64, :], bc_sb[64:128, :])

            # ---- Phase D: output projection ----
            with tc.tile_pool(name="phD", bufs=3) as pd_, \
                 tc.tile_pool(name="psD", bufs=3, space="PSUM") as psd:
                for ti in range(TT):
                    o_sb = pd_.tile([128, C], F32, tag="osb")
                    for cb in range(2):
                        pp = psd.tile([128, 512], F32, tag="pp")
                        for p in range(2):
                            nc.tensor.matmul(
                                pp,
                                yT[:, p, ti * 128:(ti + 1) * 128],
                                wp[:, p, cb * 512:(cb + 1) * 512],
                                start=(p == 0), stop=(p == 1))
                        dst = o_sb[:, cb * 512:(cb + 1) * 512]
                        if cb == 0:
                            nc.vector.tensor_copy(dst, pp)
                        else:
                            nc.scalar.copy(dst, pp)
                    nc.sync.dma_start(
                        out=out_d[ti * 128:(ti + 1) * 128, :], in_=o_sb)

    nc.compile()
    return nc


_NC = None


def _get_nc():
    global _NC
    if _NC is None:
        _NC = _build()
    return _NC


def _make_in_maps(x, w_attn, b_attn, w_proj):
    import ml_dtypes
    tri2 = np.zeros((128, 256), dtype=np.float32)
    i = np.arange(128)[:, None]
    j = np.arange(128)[None, :]
    tri = (j >= i).astype(np.float32)
    tri2[:, 0:128] = tri
    tri2[:, 128:256] = tri
    ident = np.eye(128, dtype=np.float32)
    in_maps = []
    for c in range(8):
        b = c // 4
        g = c % 4
        qs = slice(256 * g, 256 * g + 256)
        ks = slice(C + 256 * g, C + 256 * g + 256)
        vs = slice(2 * C + 256 * g, 2 * C + 256 * g + 256)
        w_local = np.concatenate(
            [w_attn[:, qs], w_attn[:, ks], w_attn[:, vs]], axis=1)
        b_local = np.concatenate([b_attn[qs], b_attn[ks], b_attn[vs]])
        wp_local = w_proj[256 * g:256 * g + 256, :]
        in_maps.append({
            "x": np.ascontiguousarray(x[b], dtype=np.float32),
            "w": np.ascontiguousarray(w_local).astype(ml_dtypes.bfloat16),
            "b": np.ascontiguousarray(b_local).astype(ml_dtypes.bfloat16),
            "wp": np.ascontiguousarray(wp_local).astype(ml_dtypes.bfloat16),
            "tri2": tri2.astype(ml_dtypes.bfloat16),
            "ident": ident,
        })
    return in_maps


def run(x, w_attn, b_attn, w_proj, b_proj, trace=False, tmpdir=None):
    x = np.asarray(x)
    w_attn = np.asarray(w_attn)
    b_attn = np.asarray(b_attn)
    w_proj = np.asarray(w_proj)
    b_proj = np.asarray(b_proj)
    nc = _get_nc()
    in_maps = _make_in_maps(x, w_attn, b_attn, w_proj)
    res = run_bass_kernel_spmd(
        nc, in_maps, core_ids=list(range(8)), trace=trace, tmpdir=tmpdir)
    parts = [res.results[c]["out"] for c in range(8)]
    out = np.empty((B, T, C), dtype=np.float32)
    for b in range(2):
        out[b] = parts[4 * b] + parts[4 * b + 1] + parts[4 * b + 2] + parts[4 * b + 3]
    out += b_proj[None, None, :].astype(np.float32)
    return out, res


def kernel(x, w_attn, b_attn, w_proj, b_proj):
    out, _ = run(x, w_attn, b_attn, w_proj, b_proj, trace=False)
    return out



# revision 17
# speedup vs baseline: 1.4645x; 1.4645x over previous
"""Causal self-attention (B=2, T=2048, C=1024, H=16) on 8 trn2 NeuronCores.

Sharding: core c handles batch b = c // 4 and head-group g = c % 4 (4 heads).
Each core computes its heads' attention and a partial output projection
(rows 256g:256g+256 of w_proj); the host sums the 4 partials per batch and
adds b_proj.

v3 vs v2:
- reciprocal_approx_fast (5x faster than DVE reciprocal; denominators >= 1).
- Startup DMAs interleaved per contraction tile so the first QKV chain
  starts ~1us in instead of waiting for the full weight/x upload.
- QKV chains of block tb+1 and output-projection chains of block tb-1 are
  emitted as fillers inside block tb's attention si-loop, so the PE stays
  busy while the Scalar engine works through the exp stream.
"""
from collections import deque

import numpy as np

import concourse.bacc as bacc
import concourse.tile as tile
import concourse.mybir as mybir
from concourse.bass_utils import run_bass_kernel_spmd

F32 = mybir.dt.float32
BF16 = mybir.dt.bfloat16

B, T, C = 2, 2048, 1024
N_HEAD = 16
D = C // N_HEAD          # 64
SCALE = D ** -0.5
HL = 4                   # heads per core (local)
CL = HL * D              # 256 local qkv feature cols per section
TT = T // 128            # 16 t-tiles of 128
TB = T // 512            # 4 t-blocks of 512
CT = C // 128            # 8 c-tiles (contraction for qkv)
VW = D + 1               # 65: v columns per head (+ ones col for denominator)


def _build():
    nc = bacc.Bacc("TRN2", debug=False)
    xT_d = nc.dram_tensor("xT", [C, T], BF16, kind="ExternalInput").ap()
    w_d = nc.dram_tensor("w", [C, 3 * CL], BF16, kind="ExternalInput").ap()
    bqk_d = nc.dram_tensor("bqk", [128, 4], F32, kind="ExternalInput").ap()
    bv_d = nc.dram_tensor("bv", [CL], BF16, kind="ExternalInput").ap()
    wp_d = nc.dram_tensor("wp", [2 * 128, C], BF16, kind="ExternalInput").ap()
    tri2_d = nc.dram_tensor("tri2", [128, 256], BF16, kind="ExternalInput").ap()
    out_d = nc.dram_tensor("out", [T, C], F32, kind="ExternalOutput").ap()

    with tile.TileContext(nc) as tc:
        with tc.tile_pool(name="persist", bufs=1) as pers:
            w_all = pers.tile([128, CT, 3 * CL], BF16)
            xT = pers.tile([128, CT, T], BF16)
            # interleave w / xT(block 0) per contraction tile g so the first
            # qk chain can start as soon as g=0 has landed
            w_r = w_d.rearrange("(g p) j -> p g j", p=128)
            x_r0 = xT_d[:, 0:512].rearrange("(g p) t -> p g t", p=128)
            for g in range(CT):
                nc.sync.dma_start(out=w_all[:, g, :], in_=w_r[:, g, :])
                nc.sync.dma_start(out=xT[:, g, 0:512], in_=x_r0[:, g, :])
            for tq in range(1, TB):
                ts = slice(tq * 512, (tq + 1) * 512)
                nc.sync.dma_start(
                    out=xT[:, :, ts],
                    in_=xT_d[:, ts].rearrange("(g p) t -> p g t", p=128))
            bqk = pers.tile([128, 4], F32)
            nc.sync.dma_start(out=bqk, in_=bqk_d)
            bv = pers.tile([1, CL], BF16)
            nc.sync.dma_start(out=bv, in_=bv_d.rearrange("(o j) -> o j", o=1))
            tri2 = pers.tile([128, 256], BF16)
            nc.sync.dma_start(out=tri2, in_=tri2_d)
            wp = pers.tile([128, 2, C], BF16)
            nc.sync.dma_start(out=wp, in_=wp_d.rearrange("(g p) j -> p g j", p=128))
            # rows 0 and 64 both hold ones: row 0 feeds the v-bias matmul,
            # row 64 feeds the denominator-broadcast matmul (whose rhs reads
            # stg row 64, and matmul operands must share a base partition)
            ones = pers.tile([128, 512], BF16)
            nc.vector.memset(ones, 1.0)

            qkT = pers.tile([128, 4, T], BF16)       # jt 0,1: q pairs; 2,3: k
            v_all = pers.tile([128, TT, HL * VW], BF16)
            yT = pers.tile([128, 2, T], BF16)
            # ones column of v_aug at col 64: denominator lands on PSUM row 64
            vones = v_all.rearrange("p t (h w) -> p t h w", h=HL)[:, :, :, D:VW]
            nc.gpsimd.memset(vones, 1.0)

            with tc.tile_pool(name="sb", bufs=3) as psb, \
                 tc.tile_pool(name="sb2", bufs=2) as psb2, \
                 tc.tile_pool(name="psAux", bufs=2, space="PSUM") as paux, \
                 tc.tile_pool(name="psST", bufs=2, space="PSUM") as pst, \
                 tc.tile_pool(name="psY", bufs=1, space="PSUM") as psy:

                def emit_qk_chain(jt, tb):
                    tsl = slice(tb * 512, (tb + 1) * 512)
                    ps = paux.tile([128, 512], F32, tag="aux", name="ps")
                    for g in range(CT):
                        nc.tensor.matmul(
                            ps,
                            w_all[:, g, jt * 128:(jt + 1) * 128],
                            xT[:, g, tsl],
                            start=(g == 0), stop=(g == CT - 1),
                            skip_group_check=True)
                    nc.vector.tensor_scalar_add(
                        qkT[:, jt, tsl], ps, bqk[:, jt:jt + 1])

                def emit_v_chain(ti):
                    psv = paux.tile([128, 512], F32, tag="aux", name="psv")
                    for g in range(CT):
                        nc.tensor.matmul(
                            psv[:, 0:CL],
                            xT[:, g, ti * 128:(ti + 1) * 128],
                            w_all[:, g, 2 * CL:3 * CL],
                            start=(g == 0), stop=False,
                            skip_group_check=True)
                    nc.tensor.matmul(
                        psv[:, 0:CL], ones[0:1, 0:128], bv,
                        start=False, stop=True, skip_group_check=True)
                    dst = v_all[:, ti, :].rearrange(
                        "p (h w) -> p h w", h=HL)[:, :, 0:D]
                    src = psv[:, 0:CL].rearrange("p (h d) -> p h d", h=HL)
                    nc.vector.tensor_copy(dst, src)

                def emit_d_tile(ti):
                    o_sb = psb.tile([128, C], F32, tag="osb", name="o_sb")
                    for cb in range(2):
                        pp = paux.tile([128, 512], F32, tag="aux", name="pp")
                        for p in range(2):
                            nc.tensor.matmul(
                                pp,
                                yT[:, p, ti * 128:(ti + 1) * 128],
                                wp[:, p, cb * 512:(cb + 1) * 512],
                                start=(p == 0), stop=(p == 1),
                                skip_group_check=True)
                        nc.vector.tensor_copy(o_sb[:, cb * 512:(cb + 1) * 512], pp)
                    nc.sync.dma_start(
                        out=out_d[ti * 128:(ti + 1) * 128, :], in_=o_sb)

                def make_fillers(tb):
                    f = deque()
                    if tb + 1 < TB:
                        for jt in range(4):
                            f.append(lambda jt=jt: emit_qk_chain(jt, tb + 1))
                        for ti in range(4 * (tb + 1), 4 * (tb + 1) + 4):
                            f.append(lambda ti=ti: emit_v_chain(ti))
                    if tb >= 1:
                        for ti in range(4 * (tb - 1), 4 * (tb - 1) + 4):
                            f.append(lambda ti=ti: emit_d_tile(ti))
                    return f

                # B(0) emitted plainly
                for jt in range(4):
                    emit_qk_chain(jt, 0)
                for ti in range(4):
                    emit_v_chain(ti)

                for tb in range(TB):
                    tsl = slice(tb * 512, (tb + 1) * 512)
                    fillers = make_fillers(tb)
                    for p in range(2):
                        n_si = 4 * (tb + 1)
                        yps = psy.tile([VW, 1024], F32, tag="yps")
                        for si in range(n_si):
                            k = si - 4 * tb
                            col0 = 128 * k if k >= 0 else 0
                            st = pst.tile([128, 1024], F32, tag="st")
                            nc.tensor.matmul(
                                st[:, col0:512],
                                qkT[0:64, 2 + p, si * 128:(si + 1) * 128],
                                qkT[0:64, p, tb * 512 + col0:(tb + 1) * 512],
                                tile_position=(0, 0), start=True, stop=True)
                            nc.tensor.matmul(
                                st[:, 512 + col0:1024],
                                qkT[64:128, 2 + p, si * 128:(si + 1) * 128],
                                qkT[64:128, p, tb * 512 + col0:(tb + 1) * 512],
                                tile_position=(64, 0), start=True, stop=True)
                            pt = psb.tile([128, 1024], BF16, tag="pt")
                            st3 = st.rearrange("p (h q) -> p h q", h=2)[:, :, col0:512]
                            pt3 = pt.rearrange("p (h q) -> p h q", h=2)[:, :, col0:512]
                            nc.scalar.activation(
                                pt3, st3, mybir.ActivationFunctionType.Exp,
                                scale=SCALE)
                            if k >= 0:
                                strip = pt.rearrange("p (h q) -> p h q", h=2)[
                                    :, :, col0:col0 + 128]
                                nc.vector.tensor_mul(
                                    strip, strip,
                                    tri2.rearrange("p (h q) -> p h q", h=2))
                            nc.tensor.matmul(
                                yps[:, col0:512],
                                v_all[:, si, 2 * p * VW:(2 * p + 1) * VW],
                                pt[:, col0:512],
                                start=(si == 0), stop=(si == n_si - 1),
                                skip_group_check=True)
                            nc.tensor.matmul(
                                yps[:, 512 + col0:1024],
                                v_all[:, si, (2 * p + 1) * VW:(2 * p + 2) * VW],
                                pt[:, 512 + col0:1024],
                                start=(si == 0), stop=(si == n_si - 1),
                                skip_group_check=True)
                            if fillers:
                                fillers.popleft()()
                        # normalization: denom -> broadcast -> recip -> mul
                        stg = psb2.tile([VW, 1024], BF16, tag="stg")
                        nc.vector.tensor_copy(stg, yps)
                        if fillers:
                            fillers.popleft()()
                        bc = paux.tile([128, 512], F32, tag="aux")
                        nc.tensor.matmul(
                            bc[0:64, :], ones[64:65, 0:64], stg[64:65, 0:512],
                            tile_position=(64, 0), start=True, stop=True)
                        nc.tensor.matmul(
                            bc[64:128, :], ones[64:65, 0:64], stg[64:65, 512:1024],
                            tile_position=(64, 64), start=True, stop=True)
                        rbc = psb2.tile([128, 512], F32, tag="rbc")
                        nc.vector.reciprocal_approx_fast(out=rbc, in_=bc)
                        nc.vector.tensor_mul(
                            yT[0:64, p, tsl], yps[0:64, 0:512], rbc[0:64, :])
                        nc.vector.tensor_mul(
                            yT[64:128, p, tsl], yps[0:64, 512:1024], rbc[64:128, :])
                    while fillers:
                        fillers.popleft()()
                # D(3)
                for ti in range(12, 16):
                    emit_d_tile(ti)

    nc.compile()
    return nc


_NC = None


def _get_nc():
    global _NC
    if _NC is None:
        _NC = _build()
    return _NC


def _make_in_maps(x, w_attn, b_attn, w_proj):
    import ml_dtypes
    tri2 = np.zeros((128, 256), dtype=np.float32)
    i = np.arange(128)[:, None]
    j = np.arange(128)[None, :]
    tri = (j >= i).astype(np.float32)
    tri2[:, 0:128] = tri
    tri2[:, 128:256] = tri
    tri2 = tri2.astype(ml_dtypes.bfloat16)
    in_maps = []
    for c in range(8):
        b = c // 4
        g = c % 4
        qs = slice(256 * g, 256 * g + 256)
        ks = slice(C + 256 * g, C + 256 * g + 256)
        vs = slice(2 * C + 256 * g, 2 * C + 256 * g + 256)
        w_local = np.concatenate(
            [w_attn[:, qs], w_attn[:, ks], w_attn[:, vs]], axis=1)
        b_local = np.concatenate([b_attn[qs], b_attn[ks], b_attn[vs]])
        bqk_local = np.ascontiguousarray(
            b_local[:512].reshape(4, 128).T.astype(np.float32))
        wp_local = w_proj[256 * g:256 * g + 256, :]
        in_maps.append({
            "xT": np.ascontiguousarray(x[b].T).astype(ml_dtypes.bfloat16),
            "w": np.ascontiguousarray(w_local).astype(ml_dtypes.bfloat16),
            "bqk": bqk_local,
            "bv": np.ascontiguousarray(b_local[512:768]).astype(ml_dtypes.bfloat16),
            "wp": np.ascontiguousarray(wp_local).astype(ml_dtypes.bfloat16),
            "tri2": tri2,
        })
    return in_maps


def run(x, w_attn, b_attn, w_proj, b_proj, trace=False, tmpdir=None):
    x = np.asarray(x)
    w_attn = np.asarray(w_attn)
    b_attn = np.asarray(b_attn)
    w_proj = np.asarray(w_proj)
    b_proj = np.asarray(b_proj)
    nc = _get_nc()
    in_maps = _make_in_maps(x, w_attn, b_attn, w_proj)
    res = run_bass_kernel_spmd(
        nc, in_maps, core_ids=list(range(8)), trace=trace, tmpdir=tmpdir)
    parts = [res.results[c]["out"] for c in range(8)]
    out = np.empty((B, T, C), dtype=np.float32)
    for b in range(2):
        out[b] = parts[4 * b] + parts[4 * b + 1] + parts[4 * b + 2] + parts[4 * b + 3]
    out += b_proj[None, None, :].astype(np.float32)
    return out, res


def kernel(x, w_attn, b_attn, w_proj, b_proj):
    out, _ = run(x, w_attn, b_attn, w_proj, b_proj, trace=False)
    return out
